# revision 42
# baseline (speedup 1.0000x reference)
"""Trainium2 Bass kernel: y = x @ weight.T + bias  (4096x4096x4096, fp32 in/out).

Sharding: 2-D (4 batch x 2 out) over the 8 NeuronCores. Core c = (bi, oi)
with bi = c // 2, oi = c % 2 computes
  y[bi*1024:(bi+1)*1024, oi*2048:(oi+1)*2048]
from x_s = x[bi*1024:+1024] (full K) and W_s = weight[oi*2048:+2048] (full K).
This halves per-core W traffic vs pure data-parallel (58.8 MB vs 84 MB HBM).

Per-core algorithm (all on device):
  - Matmuls run in bf16 (1 cyc/row on the PE at free dim 512, measured
    216 ns steady-state; psum accumulation stays fp32).
  - Both operands need K on partitions; bf16 enables the XBAR DMA
    transpose, so NO PE-transposes: the tensor engine does only the 1024
    y-matmuls + 32 K=1 bias matmuls.
    One XBAR call transposes a whole [128, 4096] row-block into
    out[p, kt, r] = in[r, kt*128+p] (verified 3-D output semantics).
  - bias is folded into PSUM with a K=1 ones-row matmul (start=True).
  - All 8 psum banks hold one o-group (512 wide), one bank per b-tile.
    Loop order (og, bt, k): a b-tile's 32 matmuls only need its own xT
    slice, and its eviction overlaps the next b-tile's matmuls.

Engine placement -- ONE queue per pipeline stage (in-order queues plus
shared stages previously caused coupling storms):
  sync (SP) + gpsimd (Pool): HBM input DMAs, alternating between the two
                 queues -- a single DGE queue only overlaps ~3 DMAs then
                 serializes with ~7us turnaround per transfer
  vector (DVE):  ALL fp32->bf16 casts (2.3us per [128,4096] row-block,
                 vs 3.7us for ACTIVATE on Act)
  scalar (Act):  ONLY the XBAR transposes -- DMA_TRANSPOSE occupies the
                 issuing queue for the whole transfer (~4.9us/[128,4096])
  vector (DVE):  also psum->sbuf y evictions (into a 4-wide batch tile --
                 every DMA pays ~8us queue turnaround regardless of size,
                 so y goes out as two 1MB DMAs per o-group, not eight 256KB)
"""
import numpy as np

import concourse.bass as bass
import concourse.mybir as mybir
import concourse.tile as tile
from concourse import bacc
from concourse.bass_utils import run_bass_kernel_spmd

F32 = mybir.dt.float32
BF16 = mybir.dt.bfloat16
P = 128

N_CORES = 8
B = 4096
K = 4096
O = 4096
BGRID = 4            # batch shards
OGRID = 2            # out shards
B_S = B // BGRID     # 1024 batch rows per core
O_S = O // OGRID     # 2048 out cols per core


def build(B_S=B_S, K=K, O_S=O_S, OG=512, n_cores=N_CORES):
    KT = K // P           # 32 k-tiles
    BT = B_S // P         # 8 b-tiles (one psum bank each)
    NOG = O_S // OG       # 4 o-groups
    OT = OG // P          # 4 o-row-blocks per o-group

    nc = bacc.Bacc("TRN2", target_bir_lowering=False, debug=False,
                   num_devices=n_cores)
    x = nc.dram_tensor("x", [B_S, K], F32, kind="ExternalInput").ap()
    w = nc.dram_tensor("w", [O_S, K], F32, kind="ExternalInput").ap()
    b = nc.dram_tensor("b", [O_S], F32, kind="ExternalInput").ap()
    y = nc.dram_tensor("y", [B_S, O_S], F32, kind="ExternalOutput").ap()

    with tile.TileContext(nc) as tc:
        with tc.tile_pool(name="const", bufs=1) as const, \
             tc.tile_pool(name="nat", bufs=2) as nat_pool, \
             tc.tile_pool(name="bf", bufs=2) as bf_pool, \
             tc.tile_pool(name="xt", bufs=1) as xt_pool, \
             tc.tile_pool(name="wt", bufs=2) as wt_pool, \
             tc.tile_pool(name="yb", bufs=1) as yb_pool, \
             tc.tile_pool(name="yps", bufs=1, space="PSUM") as yps:

            bias_f = nat_pool.tile([1, O_S], F32, tag="nat")
            nc.sync.dma_start(bias_f, b.unsqueeze(0))
            bias_sb = const.tile([1, O_S], BF16)
            nc.vector.tensor_copy(bias_sb, bias_f)
            ones_f = nat_pool.tile([1, P], F32, tag="nat")
            nc.any.memset(ones_f, 1.0)
            ones_k1 = const.tile([1, P], BF16)
            nc.vector.tensor_copy(ones_k1, ones_f)

            xT = xt_pool.tile([P, KT, B_S], BF16)   # xT[p, kt, b] = x[b, kt*P+p]

            # row-block pipeline stages: dma(sync|gpsimd, alternating so two
            # transfer streams run concurrently -- a single DGE queue only
            # overlaps ~3 DMAs then serializes with ~7us turnaround each)
            dma_q = [nc.sync, nc.gpsimd]
            qi = [0]

            def ingest_dma(src, r0):
                t_nat = nat_pool.tile([P, K], F32, tag="nat", name="t_nat")
                dma_q[qi[0] % 2].dma_start(t_nat, src[r0:r0 + P, :])
                qi[0] += 1
                return t_nat

            def ingest_castxbar(t_nat, dstT, c0):
                t_bf = bf_pool.tile([P, K], BF16, tag="bf", name="t_bf")
                nc.vector.tensor_copy(t_bf, t_nat)
                nc.scalar.dma_start(
                    dstT[:, :, c0:c0 + P], t_bf, transpose=True)

            def x_ingest(bt):
                ingest_castxbar(ingest_dma(x, bt * P), xT, bt * P)

            def x_ingest0_split():
                # b-tile 0 gates the first matmul: fetch its two K-halves on
                # both lanes concurrently, cast into one bf tile, one xbar.
                halves = []
                for h in range(2):
                    t_nat = nat_pool.tile([P, K // 2], F32, tag="nat",
                                          name="x0h")
                    dma_q[h].dma_start(
                        t_nat, x[0:P, h * (K // 2):(h + 1) * (K // 2)])
                    halves.append(t_nat)
                t_bf = bf_pool.tile([P, K], BF16, tag="bf", name="x0bf")
                for h in range(2):
                    nc.vector.tensor_copy(
                        t_bf[:, h * (K // 2):(h + 1) * (K // 2)], halves[h])
                nc.scalar.dma_start(xT[:, :, 0:P], t_bf, transpose=True)

            wT = [None] * NOG
            # Priming: x b-tile 0 first (its xbar gates the very first
            # matmuls), then W o-group 0 (gates everything), then the
            # remaining x b-tiles in use order.
            x_ingest0_split()
            wT[0] = wt_pool.tile([P, KT, OG], BF16, name="wT0", tag="wT")
            for ot in range(OT):
                ingest_castxbar(ingest_dma(w, ot * P), wT[0], ot * P)
            for bt in range(1, BT):
                x_ingest(bt)

            for og in range(NOG):
                prefetch = og + 1 < NOG
                w_nats = []
                if prefetch:
                    wT[og + 1] = wt_pool.tile([P, KT, OG], BF16,
                                              name=f"wT{og + 1}", tag="wT")
                    # 2 of 4 DMAs issued up-front (nat pool holds 2); the
                    # rest go out as casts free slots.
                    for ot in range(2):
                        w_nats.append(ingest_dma(w, (og + 1) * OG + ot * P))

                yb = yb_pool.tile([P, BT, OG], F32, name=f"yb{og}", tag="yb")
                # o-group 0 sweeps in two 256-wide halves: wT reads are
                # tracked at slice level, so the first half's matmuls need
                # only the first two W row-block xbars -- the PE starts
                # ~30us earlier instead of waiting for all four.
                if og == 0:
                    sections = [(bt, h * (OG // 2), OG // 2)
                                for h in range(2) for bt in range(BT)]
                else:
                    sections = [(bt, 0, OG) for bt in range(BT)]

                for si, (bt, o_lo, o_w) in enumerate(sections):
                    if prefetch and si in (1, 3):
                        w_nats.append(
                            ingest_dma(w, (og + 1) * OG + (si // 2 + 2) * P))
                    # Interleave the next o-group's cast+xbar into the middle
                    # of this o-group's compute, after their DMAs landed.
                    if prefetch and 2 <= si < 2 + OT:
                        ingest_castxbar(w_nats[si - 2], wT[og + 1],
                                        (si - 2) * P)

                    psum_y = yps.tile([P, o_w], F32,
                                      name=f"psum_y{og}_{si}",
                                      tag=f"psum_y{bt}")
                    nc.tensor.matmul(
                        psum_y, ones_k1,
                        bias_sb[:, og * OG + o_lo:og * OG + o_lo + o_w],
                        start=True, stop=False)
                    for k in range(KT):
                        nc.tensor.matmul(
                            psum_y,
                            xT[:, k, bt * P:(bt + 1) * P],
                            wT[og][:, k, o_lo:o_lo + o_w],
                            start=False,
                            stop=(k == KT - 1),
                        )
                    nc.vector.tensor_copy(yb[:, bt, o_lo:o_lo + o_w], psum_y)
                # ONE y DMA per o-group (every DMA costs a ~20us lane slot:
                # ~12us transfer + ~8us queue turnaround), lane alternating.
                dma_q[og % 2].dma_start(
                    y[:, og * OG:(og + 1) * OG]
                    .rearrange("(q p) o -> p q o", p=P),
                    yb)
                wT[og] = None

    nc.compile()
    return nc


_nc_cache = {}


def get_nc():
    if "nc" not in _nc_cache:
        _nc_cache["nc"] = build()
    return _nc_cache["nc"]


def make_in_maps(x, weight, bias):
    x = np.ascontiguousarray(np.asarray(x, dtype=np.float32))
    weight = np.ascontiguousarray(np.asarray(weight, dtype=np.float32))
    bias = np.ascontiguousarray(np.asarray(bias, dtype=np.float32))
    assert x.shape == (B, K) and weight.shape == (O, K) and bias.shape == (O,)
    maps = []
    for c in range(N_CORES):
        bi, oi = c // OGRID, c % OGRID
        maps.append({
            "x": np.ascontiguousarray(x[bi * B_S:(bi + 1) * B_S]),
            "w": np.ascontiguousarray(weight[oi * O_S:(oi + 1) * O_S]),
            "b": np.ascontiguousarray(bias[oi * O_S:(oi + 1) * O_S]),
        })
    return maps


def run(x, weight, bias, **spmd_kwargs):
    """Run on all 8 cores; returns (y_full, BassKernelResults)."""
    nc = get_nc()
    in_maps = make_in_maps(x, weight, bias)
    res = run_bass_kernel_spmd(nc, in_maps, list(range(N_CORES)), **spmd_kwargs)
    y_full = np.empty((B, O), dtype=np.float32)
    for c in range(N_CORES):
        bi, oi = c // OGRID, c % OGRID
        y_full[bi * B_S:(bi + 1) * B_S, oi * O_S:(oi + 1) * O_S] = \
            res.results[c]["y"]
    return y_full, res


def kernel(x, weight, bias):
    y, _ = run(x, weight, bias)
    return y


# revision 44
# speedup vs baseline: 1.0262x; 1.0262x over previous
"""Trainium2 Bass kernel: y = x @ weight.T + bias  (4096x4096x4096, fp32 in/out).

Sharding: 2-D (4 batch x 2 out) over the 8 NeuronCores. Core c = (bi, oi)
with bi = c // 2, oi = c % 2 computes
  y[bi*1024:(bi+1)*1024, oi*2048:(oi+1)*2048]
from x_s = x[bi*1024:+1024] (full K) and W_s = weight[oi*2048:+2048] (full K).
This halves per-core W traffic vs pure data-parallel (58.8 MB vs 84 MB HBM).

Per-core algorithm (all on device):
  - Matmuls run in bf16 (1 cyc/row on the PE at free dim 512, measured
    216 ns steady-state; psum accumulation stays fp32).
  - Both operands need K on partitions; bf16 enables the XBAR DMA
    transpose, so NO PE-transposes: the tensor engine does only the 1024
    y-matmuls + 32 K=1 bias matmuls.
    One XBAR call transposes a whole [128, 4096] row-block into
    out[p, kt, r] = in[r, kt*128+p] (verified 3-D output semantics).
  - bias is folded into PSUM with a K=1 ones-row matmul (start=True).
  - All 8 psum banks hold one o-group (512 wide), one bank per b-tile.
    Loop order (og, bt, k): a b-tile's 32 matmuls only need its own xT
    slice, and its eviction overlaps the next b-tile's matmuls.

Engine placement -- ONE queue per pipeline stage (in-order queues plus
shared stages previously caused coupling storms):
  sync (SP) + gpsimd (Pool): HBM input DMAs, alternating between the two
                 queues -- a single DGE queue only overlaps ~3 DMAs then
                 serializes with ~7us turnaround per transfer
  vector (DVE):  ALL fp32->bf16 casts (2.3us per [128,4096] row-block,
                 vs 3.7us for ACTIVATE on Act)
  scalar (Act):  ONLY the XBAR transposes -- DMA_TRANSPOSE occupies the
                 issuing queue for the whole transfer (~4.9us/[128,4096])
  vector (DVE):  also psum->sbuf y evictions (into a 4-wide batch tile --
                 every DMA pays ~8us queue turnaround regardless of size,
                 so y goes out as two 1MB DMAs per o-group, not eight 256KB)
"""
import numpy as np

import concourse.bass as bass
import concourse.mybir as mybir
import concourse.tile as tile
from concourse import bacc
from concourse.bass_utils import run_bass_kernel_spmd

F32 = mybir.dt.float32
BF16 = mybir.dt.bfloat16
P = 128

N_CORES = 8
B = 4096
K = 4096
O = 4096
BGRID = 4            # batch shards
OGRID = 2            # out shards
B_S = B // BGRID     # 1024 batch rows per core
O_S = O // OGRID     # 2048 out cols per core


def build(B_S=B_S, K=K, O_S=O_S, OG=512, n_cores=N_CORES):
    KT = K // P           # 32 k-tiles
    BT = B_S // P         # 8 b-tiles (one psum bank each)
    NOG = O_S // OG       # 4 o-groups
    OT = OG // P          # 4 o-row-blocks per o-group

    nc = bacc.Bacc("TRN2", target_bir_lowering=False, debug=False,
                   num_devices=n_cores)
    x = nc.dram_tensor("x", [B_S, K], F32, kind="ExternalInput").ap()
    w = nc.dram_tensor("w", [O_S, K], F32, kind="ExternalInput").ap()
    b = nc.dram_tensor("b", [O_S], F32, kind="ExternalInput").ap()
    y = nc.dram_tensor("y", [B_S, O_S], F32, kind="ExternalOutput").ap()

    with tile.TileContext(nc) as tc:
        with tc.tile_pool(name="const", bufs=1) as const, \
             tc.tile_pool(name="nat", bufs=2) as nat_pool, \
             tc.tile_pool(name="bf", bufs=2) as bf_pool, \
             tc.tile_pool(name="xt", bufs=1) as xt_pool, \
             tc.tile_pool(name="wt", bufs=2) as wt_pool, \
             tc.tile_pool(name="yb", bufs=1) as yb_pool, \
             tc.tile_pool(name="yps", bufs=1, space="PSUM") as yps:

            bias_f = nat_pool.tile([1, O_S], F32, tag="nat")
            nc.sync.dma_start(bias_f, b.unsqueeze(0))
            bias_sb = const.tile([1, O_S], BF16)
            nc.vector.tensor_copy(bias_sb, bias_f)
            ones_f = nat_pool.tile([1, P], F32, tag="nat")
            nc.any.memset(ones_f, 1.0)
            ones_k1 = const.tile([1, P], BF16)
            nc.vector.tensor_copy(ones_k1, ones_f)

            xT = xt_pool.tile([P, KT, B_S], BF16)   # xT[p, kt, b] = x[b, kt*P+p]

            # row-block pipeline stages: dma(sync|gpsimd, alternating so two
            # transfer streams run concurrently -- a single DGE queue only
            # overlaps ~3 DMAs then serializes with ~7us turnaround each)
            dma_q = [nc.sync, nc.gpsimd]
            qi = [0]

            def ingest_dma(src, r0):
                t_nat = nat_pool.tile([P, K], F32, tag="nat", name="t_nat")
                dma_q[qi[0] % 2].dma_start(t_nat, src[r0:r0 + P, :])
                qi[0] += 1
                return t_nat

            def ingest_castxbar(t_nat, dstT, c0):
                t_bf = bf_pool.tile([P, K], BF16, tag="bf", name="t_bf")
                nc.vector.tensor_copy(t_bf, t_nat)
                nc.scalar.dma_start(
                    dstT[:, :, c0:c0 + P], t_bf, transpose=True)

            def x_ingest(bt):
                ingest_castxbar(ingest_dma(x, bt * P), xT, bt * P)

            def x_ingest0_split():
                # b-tile 0 gates the first matmul: fetch its two K-halves on
                # both lanes concurrently, cast into one bf tile, one xbar.
                halves = []
                for h in range(2):
                    t_nat = nat_pool.tile([P, K // 2], F32, tag="nat",
                                          name="x0h")
                    dma_q[h].dma_start(
                        t_nat, x[0:P, h * (K // 2):(h + 1) * (K // 2)])
                    halves.append(t_nat)
                t_bf = bf_pool.tile([P, K], BF16, tag="bf", name="x0bf")
                for h in range(2):
                    nc.vector.tensor_copy(
                        t_bf[:, h * (K // 2):(h + 1) * (K // 2)], halves[h])
                nc.scalar.dma_start(xT[:, :, 0:P], t_bf, transpose=True)

            wT = [None] * NOG
            # Priming: x b-tile 0 first (its xbar gates the very first
            # matmuls), then W o-group 0 (gates everything), then the
            # remaining x b-tiles in use order.
            x_ingest0_split()
            wT[0] = wt_pool.tile([P, KT, OG], BF16, name="wT0", tag="wT")
            for ot in range(OT):
                ingest_castxbar(ingest_dma(w, ot * P), wT[0], ot * P)
            for bt in range(1, BT):
                x_ingest(bt)

            for og in range(NOG):
                prefetch = og + 1 < NOG
                w_nats = []
                if prefetch:
                    wT[og + 1] = wt_pool.tile([P, KT, OG], BF16,
                                              name=f"wT{og + 1}", tag="wT")
                    # 2 of 4 DMAs issued up-front (nat pool holds 2); the
                    # rest go out as casts free slots.
                    for ot in range(2):
                        w_nats.append(ingest_dma(w, (og + 1) * OG + ot * P))

                yb = yb_pool.tile([P, BT, OG], F32, name=f"yb{og}", tag="yb")
                for bt in range(BT):
                    if prefetch and bt in (1, 3):
                        w_nats.append(
                            ingest_dma(w, (og + 1) * OG + (bt // 2 + 2) * P))
                    # Interleave the next o-group's cast+xbar into the middle
                    # of this o-group's compute, after their DMAs landed.
                    if prefetch and 2 <= bt < 2 + OT:
                        ingest_castxbar(w_nats[bt - 2], wT[og + 1],
                                        (bt - 2) * P)

                    psum_y = yps.tile([P, OG], F32, name=f"psum_y{og}_{bt}",
                                      tag=f"psum_y{bt}")
                    nc.tensor.matmul(
                        psum_y, ones_k1,
                        bias_sb[:, og * OG:(og + 1) * OG],
                        start=True, stop=False)
                    for k in range(KT):
                        nc.tensor.matmul(
                            psum_y,
                            xT[:, k, bt * P:(bt + 1) * P],
                            wT[og][:, k, :],
                            start=False,
                            stop=(k == KT - 1),
                        )
                    nc.vector.tensor_copy(yb[:, bt, :], psum_y)
                # ONE y DMA per o-group (every DMA costs a ~20us lane slot:
                # ~12us transfer + ~8us queue turnaround), issued on the Act
                # queue -- it has slack between xbars, and this keeps all
                # input-lane slots for the W/x stream.
                nc.scalar.dma_start(
                    y[:, og * OG:(og + 1) * OG]
                    .rearrange("(q p) o -> p q o", p=P),
                    yb)
                wT[og] = None

    nc.compile()
    return nc


_nc_cache = {}


def get_nc():
    if "nc" not in _nc_cache:
        _nc_cache["nc"] = build()
    return _nc_cache["nc"]


def make_in_maps(x, weight, bias):
    x = np.ascontiguousarray(np.asarray(x, dtype=np.float32))
    weight = np.ascontiguousarray(np.asarray(weight, dtype=np.float32))
    bias = np.ascontiguousarray(np.asarray(bias, dtype=np.float32))
    assert x.shape == (B, K) and weight.shape == (O, K) and bias.shape == (O,)
    maps = []
    for c in range(N_CORES):
        bi, oi = c // OGRID, c % OGRID
        maps.append({
            "x": np.ascontiguousarray(x[bi * B_S:(bi + 1) * B_S]),
            "w": np.ascontiguousarray(weight[oi * O_S:(oi + 1) * O_S]),
            "b": np.ascontiguousarray(bias[oi * O_S:(oi + 1) * O_S]),
        })
    return maps


def run(x, weight, bias, **spmd_kwargs):
    """Run on all 8 cores; returns (y_full, BassKernelResults)."""
    nc = get_nc()
    in_maps = make_in_maps(x, weight, bias)
    res = run_bass_kernel_spmd(nc, in_maps, list(range(N_CORES)), **spmd_kwargs)
    y_full = np.empty((B, O), dtype=np.float32)
    for c in range(N_CORES):
        bi, oi = c // OGRID, c % OGRID
        y_full[bi * B_S:(bi + 1) * B_S, oi * O_S:(oi + 1) * O_S] = \
            res.results[c]["y"]
    return y_full, res


def kernel(x, weight, bias):
    y, _ = run(x, weight, bias)
    return y
